# revision 62
# baseline (speedup 1.0000x reference)
"""Trainium2 Bass kernel for a ViT-style attention block + classifier head.

Reference computation (per batch b of 4, N=2048 tokens, C=768, 12 heads x 64):
    qkv  = x @ w_qkv                         [B,N,3C]
    attn = softmax(q k^T / 8)                per head
    out  = (attn @ v) reassembled            [B,N,C]
    out  = out @ w_proj + b_proj
    out  = out @ w_head + b_head             [B,N,1000]
    return max over N                        [B,1000]

Sharding: 8 cores = 4 batches x 2 query-halves (1024 queries each).
Each core computes K/V for its full batch (duplicated within the pair),
attention for its query half, then a fused (w_proj @ w_head) classifier
matmul and a local max over its 1024 queries -> [1000] per core.
Host combines with np.maximum and adds the fused bias afterwards
(max is invariant to adding a per-row constant).

Key performance structure (cost-model driven):
  * Everything on the q/k side of softmax runs in fp8e4 DoubleRow matmuls
    (0.5 cycles/row): the q/k projections contract fp8 x against fp8
    (8x-prescaled) weights, and the scores matmuls contract fp8-evacuated
    q/k. DoubleRow's second contraction lane points at zeroed/padded
    column blocks so no partition repacking is needed; the 1/64 prescale
    correction folds into the exp activation scale. fp8 noise only
    perturbs the exp argument (damped by the 1/8 softmax scale) - the
    value path (v, attn@v, classifier) stays bf16.
  * attn@v is computed transposed: out[q, d] accumulates over key chunks in
    PSUM with only 65 streamed columns per matmul (64 v-dims + a ones column
    that yields the softmax denominator), queries on the partition axis.
    16 accumulators (2 heads x 8 query chunks) pack into 3 PSUM banks as
    130-column slots; the first matmul in each bank carries start=True
    (PSUM zero regions are bank-granular), the rest rely on the lazy
    bank-wide zero.
  * The whole kernel is emitted as one software-pipelined stream: attn@v
    matmuls trail their scores/exp by LAG key-chunks so that by the time
    they reach the PE wait queue their exp dependency is already satisfied
    (the 4-deep in-order wait queue otherwise blocks the PE sequencer),
    fillers (v-production, next pairs' q/k projection) spread across the
    preceding pairs' steps, and normalize/transpose work for a pair rides
    the next pair's steps.
  * Exps run 1024-wide on ScalarE from 2-bank PSUM score tiles; ScalarE is
    the bottleneck engine, so everything else hides behind it.  Input DMAs
    are batched into a few strided transfers (HWDGE setup is ~625ns per
    dma_start) and a warmup matmul spree ramps the PE out of its low
    p-state before the real lead-in work arrives.
"""

import sys

for _p in ("/opt/trn_rl_repo", "/root/.axon_site/_ro/trn_rl_repo"):
    if _p not in sys.path:
        sys.path.append(_p)

import numpy as np
import ml_dtypes

import concourse.bacc as bacc
import concourse.mybir as mybir
from concourse.alu_op_type import AluOpType
from concourse.tile import TileContext
from concourse.bass_utils import run_bass_kernel_spmd

BF16 = mybir.dt.bfloat16
FP8 = mybir.dt.float8e4
F32 = mybir.dt.float32
I16 = mybir.dt.int16
DR = mybir.MatmulPerfMode.DoubleRow

B, N, C = 4, 2048, 768
HEADS, HD = 12, 64
NUM_CLASSES = 1000
SCALE = HD ** (-0.5)
QK_PRESCALE = 8.0          # host-folded into the fp8 q/k weights

NQ = 1024          # queries per core
KC = N // 128      # 16 key chunks
CC = C // 128      # 6 contraction chunks
PAIRS = HEADS // 2  # head-pair tiles (2 x 64 partitions)
NCLS = NUM_CLASSES
LAG = 3            # attn@v trails scores/exp by this many key chunks
VG = 2             # v-production group width in pairs

_CACHE = {}
EMIT_LOG = []   # (approx first instruction index, label) for trace attribution


def _build():
    nc = bacc.Bacc("TRN2", target_bir_lowering=False)

    # xT arrives key-rotated per core so that columns 0:NQ are always this
    # core's query rows (attention is invariant to key order; the final max
    # is invariant to query order).
    xT_d = nc.dram_tensor("xT", [C, N], BF16, kind="ExternalInput")
    xT8_d = nc.dram_tensor("xT8", [C, N], FP8, kind="ExternalInput")
    # [vals | zeros] per row: the host ships the DoubleRow zero lane
    wq8_d = nc.dram_tensor("wq8", [C, 2 * C], FP8, kind="ExternalInput")
    wk8_d = nc.dram_tensor("wk8", [C, 2 * C], FP8, kind="ExternalInput")
    wv_d = nc.dram_tensor("wv", [C, C], BF16, kind="ExternalInput")
    wf_d = nc.dram_tensor("wf", [C, NCLS], BF16, kind="ExternalInput")
    ident_d = nc.dram_tensor("ident", [128, 128], F32, kind="ExternalInput")
    identb_d = nc.dram_tensor("identb", [128, 128], BF16, kind="ExternalInput")
    out_d = nc.dram_tensor("out", [128, NCLS], F32, kind="ExternalOutput")

    EXP = mybir.ActivationFunctionType.Exp
    ESCALE = SCALE / (QK_PRESCALE * QK_PRESCALE)
    # Schraudolph exp on DVE for a subset of key chunks: y = scores*A16 +
    # B16 truncated to int16 IS the bf16 bit pattern of ~exp(scores*ESCALE)
    # (~1.8% rms), one tensor_scalar op per half. Disabled: routing these
    # chunks through the single-buffer staging bank serializes against the
    # other staging users and loses more than the ScalarE chain saves;
    # bypassing the 2-tile score rotation profitably needs a third rotating
    # score tile, which the 8-bank PSUM budget does not admit.
    SCHRAU_KC = ()
    A16 = (2.0 ** 23 / np.log(2.0)) * ESCALE / 65536.0
    B16 = 1064900000.0 / 65536.0

    with TileContext(nc) as tc:
        with (
            tc.tile_pool(name="wpool", bufs=1) as wpool,
            tc.tile_pool(name="xpool", bufs=1) as xpool,
            tc.tile_pool(name="qk8p", bufs=1) as qk8p,
            tc.tile_pool(name="vp", bufs=1) as vp,
            tc.tile_pool(name="ep", bufs=10) as ep,
            tc.tile_pool(name="outp", bufs=1) as outp,
            tc.tile_pool(name="small", bufs=8) as smallp,
            tc.tile_pool(name="lg", bufs=1) as lgp,
            # PSUM: 2 x [128,1024] rotating score tiles (4 banks) reserved
            # for the scores->exp double buffer, 3 x [128,512] attn@v
            # accumulator banks, and 1 staging bank shared (in 512-col
            # chunks) by v-production, q/k projection halves, and the
            # f32 transposes.
            tc.tile_pool(name="ps", bufs=2, space="PSUM") as psp,
            tc.tile_pool(name="av", bufs=3, space="PSUM") as avp,
            tc.tile_pool(name="stage", bufs=1, space="PSUM") as stgp,
        ):
            # ---- static SBUF tiles (one big tile per tensor so input DMAs
            # batch into a handful of strided transfers) ----
            xTb = xpool.tile([128, CC * N], BF16, tag="xT", name="xT_sb", bufs=1)
            # fp8 x with a single 512-col tail pad (the DoubleRow junk lane
            # of the last chunk's last half reads past the end; the weight
            # zero lane annihilates it, it just has to be initialized)
            xT8b = xpool.tile([128, CC * N + 512], FP8, tag="xT8", name="xT8_sb", bufs=1)
            wq8b = wpool.tile([128, CC * 2 * C], FP8, tag="wq8", name="wq8_sb", bufs=1)
            wk8b = wpool.tile([128, CC * 2 * C], FP8, tag="wk8", name="wk8_sb", bufs=1)
            wvb = wpool.tile([128, CC * C], BF16, tag="wv", name="wv_sb", bufs=1)
            wfb = wpool.tile([128, CC * NCLS], BF16, tag="wf", name="wf_sb", bufs=1)
            ident = wpool.tile([128, 128], F32, tag="ident", name="ident_sb", bufs=1)
            identb = wpool.tile([128, 128], BF16, tag="identb", name="identb_sb", bufs=1)
            xT = [xTb[:, c * N:(c + 1) * N] for c in range(CC)]
            wv = [wvb[:, c * C:(c + 1) * C] for c in range(CC)]
            wf = [wfb[:, c * NCLS:(c + 1) * NCLS] for c in range(CC)]
            # Per pair fp8 q/k: cols 0:N k values, N:N+NQ q values,
            # N+NQ:N+2NQ q zero lane. Only the rhs (q) side needs a real
            # zero lane - the k-side junk lane multiplies it.
            qk8 = [qk8p.tile([128, N + 2 * NQ], FP8, tag="qk8", name="qk8_sb", bufs=PAIRS)
                   for _ in range(PAIRS)]
            # v with a ones column appended per head: [128, 12*65]
            v65 = [vp.tile([128, HEADS * (HD + 1)], BF16, tag="v65", name="v65_sb", bufs=KC)
                   for _ in range(KC)]
            # normalized attention output, queries on partitions: col h*64+d
            # (f32 so the PE transpose can share the f32 staging bank)
            out_qc = [outp.tile([128, C], F32, tag="oqc", name="oqc_sb", bufs=NQ // 128)
                      for _ in range(NQ // 128)]
            # transposed back: channels on partitions
            outT = [outp.tile([128, NQ], BF16, tag="outT", name="outT_sb", bufs=CC) for _ in range(CC)]
            # classifier partial sums (contraction chunks 0..3), bf16
            pcl = [outp.tile([128, NCLS], BF16, tag="pcl", name="pcl_sb", bufs=NQ // 128)
                   for _ in range(NQ // 128)]
            # two interleaved max accumulators (breaks the serial DVE chain)
            lgmax = [lgp.tile([128, NCLS], F32, tag="lgmax", name="lgmax_sb",
                              bufs=2)
                     for _ in range(2)]

            # ---- PE warmup: ~3.5us of back-to-back dummy matmuls ramps the
            # p-state (alternating psp banks so no WAW gap resets the ramp;
            # uses the wf region as a scratch source, its DMA comes later)
            nc.vector.memset(wfb[:, 0:512], 0.25)
            for i in range(4):
                ws = psp.tile([128, 1024], F32, tag="ps", name="ps")
                for s0 in (0, 512):
                    nc.tensor.matmul(ws[:, s0:s0 + 512], lhsT=wfb[:, 0:128],
                                     rhs=wfb[:, 0:512], start=True, stop=True)

            # ---- input DMA (batched over parallel engine queues; ordered
            # for the pipeline lead-in) ----
            x8_src = xT8_d[:].rearrange("(c p) n -> p c n", p=128)
            x8_dst = xT8b[:, 0:CC * N].rearrange("p (c n) -> p c n", n=N)
            xT_src = xT_d[:].rearrange("(c p) n -> p c n", p=128)
            xT_dst = xTb[:].rearrange("p (c n) -> p c n", n=N)
            wv_src = wv_d[:].rearrange("(c p) m -> p c m", p=128)
            wv_dst = wvb[:].rearrange("p (c m) -> p c m", m=C)

            # pair 0's DoubleRow q zero lane + the xT8 tail pad come first on
            # the Pool engine - they gate the first scores
            nc.gpsimd.memset(qk8[0][:, N + NQ:], 0.0)
            nc.gpsimd.memset(xT8b[:, CC * N:], 0.0)

            # (a) fp8 x halves on the SP queue; fp8 q/k weights (with
            # host-shipped zero lanes) in parallel on the ACT queue
            nc.sync.dma_start(out=x8_dst[:, :, 0:NQ], in_=x8_src[:, :, 0:NQ])
            nc.sync.dma_start(out=x8_dst[:, :, NQ:N], in_=x8_src[:, :, NQ:N])
            nc.scalar.dma_start(
                out=wq8b[:].rearrange("p (c m) -> p c m", m=2 * C),
                in_=wq8_d[:].rearrange("(c p) m -> p c m", p=128))
            nc.scalar.dma_start(
                out=wk8b[:].rearrange("p (c m) -> p c m", m=2 * C),
                in_=wk8_d[:].rearrange("(c p) m -> p c m", p=128))
            nc.gpsimd.dma_start(out=ident[:], in_=ident_d[:, :])
            nc.gpsimd.dma_start(out=identb[:], in_=identb_d[:, :])
            # (b) bf16 xT query-half + group-0 v weights (v-production for
            # the first 8 key chunks only needs the query-half columns)
            nc.sync.dma_start(out=xT_dst[:, :, 0:NQ], in_=xT_src[:, :, 0:NQ])
            nc.sync.dma_start(out=wv_dst[:, :, 0:VG * 128],
                              in_=wv_src[:, :, 0:VG * 128])
            nc.sync.dma_start(out=xT_dst[:, :, NQ:N], in_=xT_src[:, :, NQ:N])
            # (c) remaining v weights
            nc.sync.dma_start(out=wv_dst[:, :, VG * 128:],
                              in_=wv_src[:, :, VG * 128:])
            # (d) classifier weights
            nc.sync.dma_start(
                out=wfb[:].rearrange("p (c m) -> p c m", m=NCLS),
                in_=wf_d[:].rearrange("(c p) m -> p c m", p=128))

            # remaining pairs' q zero lanes
            for p in range(1, PAIRS):
                nc.gpsimd.memset(qk8[p][:, N + NQ:], 0.0)

            # ---- unit emitters ----
            def x8_rhs(c, s0):
                # [128, 2, 512] fp8 view: lane 0 = keys s0..s0+512, lane 1 =
                # the next 512 cols (tail pad for the very last half;
                # annihilated by the zero weight lane)
                return xT8b[:, c * N + s0:c * N + s0 + 1024].rearrange(
                    "p (i n) -> p i n", i=2)

            def w8_lhsT(wb, c, p):
                return wb[:, c * 2 * C:(c + 1) * 2 * C].rearrange(
                    "p (i m) -> p i m", i=2)[:, :, p * 128:(p + 1) * 128]

            def proj_mms(dst, wb, p, s0, src_s0):
                for c in range(CC):
                    nc.tensor.matmul(
                        dst, lhsT=w8_lhsT(wb, c, p), rhs=x8_rhs(c, src_s0),
                        start=(c == 0), stop=(c == CC - 1), perf_mode=DR)

            def q_half(p, s0):
                ps = stgp.tile([128, 512], F32, tag="stage", name="stage")
                proj_mms(ps[:], wq8b, p, s0, s0)
                nc.vector.tensor_copy(out=qk8[p][:, N + s0:N + s0 + 512], in_=ps[:])

            def k_half(p, s0):
                ps = stgp.tile([128, 512], F32, tag="stage", name="stage")
                proj_mms(ps[:], wk8b, p, s0, s0)
                nc.vector.tensor_copy(out=qk8[p][:, s0:s0 + 512], in_=ps[:])

            def qk_halves(p):
                return ([lambda p=p, s0=s0: q_half(p, s0) for s0 in (0, 512)]
                        + [lambda p=p, s0=s0: k_half(p, s0)
                           for s0 in (0, 512, 1024, 1536)])

            def qk_unit(p, which, u):
                # lead-in variant: full [128,1024] through a psp tile
                ps = psp.tile([128, 1024], F32, tag="ps", name="ps")
                wb = wq8b if which == "q" else wk8b
                for s0 in (0, 512):
                    for c in range(CC):
                        nc.tensor.matmul(
                            ps[:, s0:s0 + 512], lhsT=w8_lhsT(wb, c, p),
                            rhs=x8_rhs(c, u * 1024 + s0),
                            start=(c == 0), stop=(c == CC - 1), perf_mode=DR)
                off = N if which == "q" else u * 1024
                nc.vector.tensor_copy(out=qk8[p][:, off:off + 1024], in_=ps[:])

            def v_group(g, kc):
                """v65[kc] columns for pairs VG*g..VG*g+VG-1 (+ ones)."""
                w0 = g * VG * 2 * HD
                psb = stgp.tile([128, 512], F32, tag="stage", name="stage")
                ps = psb[:, 0:VG * 2 * HD]
                for c in range(CC):
                    nc.tensor.matmul(
                        ps[:], lhsT=xT[c][:, kc * 128:(kc + 1) * 128],
                        rhs=wv[c][:, w0:w0 + VG * 2 * HD],
                        start=(c == 0), stop=(c == CC - 1))
                vdst = v65[kc][:].rearrange("p (h d) -> p h d", d=HD + 1)
                hs = slice(g * VG * 2, (g + 1) * VG * 2)
                nc.vector.memset(vdst[:, hs, HD:HD + 1], 1.0)
                nc.vector.tensor_copy(
                    out=vdst[:, hs, 0:HD],
                    in_=ps[:].rearrange("p (h d) -> p h d", d=HD))

            es = {}     # (p, kc) -> (e0, e1)
            avb = {}    # p -> [3 psum bank tiles]

            def scores_step(p, kc):
                # k lane 1 reads the q/qzero region (finite junk, multiplied
                # by q's zero lane); q lane 1 is the real zero lane.
                kv = qk8[p][:, 0:2 * N].rearrange("p (i n) -> p i n", i=2)
                qv = qk8[p][:, N:N + 2 * NQ].rearrange("p (i n) -> p i n", i=2)
                pair_es = []
                for h in (0, 1):
                    rows = slice(h * HD, (h + 1) * HD)
                    e = ep.tile([128, 1024], BF16, tag="e", name="e")
                    if kc in SCHRAU_KC:
                        # Schraudolph exp on DVE through the staging bank:
                        # these chunks leave the psp rotation entirely, so
                        # ScalarE's chain skips them and runs ahead while
                        # DVE consumes in parallel.
                        for s0 in (0, 512):
                            st = stgp.tile([128, 512], F32, tag="stage",
                                           name="stage")
                            nc.tensor.matmul(
                                st[:],
                                lhsT=kv[rows, :, kc * 128:(kc + 1) * 128],
                                rhs=qv[rows, :, s0:s0 + 512],
                                start=True, stop=True, perf_mode=DR)
                            nc.vector.tensor_scalar(
                                out=e[:, s0:s0 + 512].bitcast(I16), in0=st[:],
                                scalar1=A16, scalar2=B16,
                                op0=AluOpType.mult, op1=AluOpType.add)
                    else:
                        st = psp.tile([128, 1024], F32, tag="ps", name="ps")
                        for s0 in (0, 512):
                            nc.tensor.matmul(
                                st[:, s0:s0 + 512],
                                lhsT=kv[rows, :, kc * 128:(kc + 1) * 128],
                                rhs=qv[rows, :, s0:s0 + 512],
                                start=True, stop=True, perf_mode=DR)
                        nc.scalar.activation(out=e[:], in_=st[:], func=EXP,
                                             scale=ESCALE)
                    pair_es.append(e)
                es[(p, kc)] = pair_es

            # av accumulator geometry: 8 slots of 130 cols over 3 banks
            # bank 0: qc 0..2, bank 1: qc 3..5, bank 2: qc 6..7
            def av_slot(qc):
                return qc // 3, (qc % 3) * 130

            def av_step(p, kc):
                if kc == 0:
                    avb[p] = [avp.tile([128, 512], F32, tag="av", name="av")
                              for _ in range(3)]
                banks = avb[p]
                pair_es = es.pop((p, kc))
                for qc in range(NQ // 128):
                    bnk, col = av_slot(qc)
                    for h in (0, 1):
                        first = kc == 0 and col == 0 and h == 0
                        last = (kc == KC - 1 and h == 1
                                and (qc % 3 == 2 or qc == NQ // 128 - 1))
                        nc.tensor.matmul(
                            banks[bnk][:, col + h * 65:col + h * 65 + 65],
                            lhsT=pair_es[h][:, qc * 128:(qc + 1) * 128],
                            rhs=v65[kc][:, (2 * p + h) * 65:(2 * p + h) * 65 + 65],
                            start=first, stop=last,
                            skip_group_check=not (first or last))

            def norm(p, qc, on_act=False):
                # on_act: kernel tail - ScalarE is idle there, DVE is not
                bnk, col = av_slot(qc)
                banks = avb[p]
                r = smallp.tile([128, 2], F32, tag="rcp", name="rcp")
                nc.vector.reciprocal_approx_fast(
                    out=r[:], in_=banks[bnk][:, col + 64:col + 130:65])
                COPY = mybir.ActivationFunctionType.Copy
                for h in (0, 1):
                    dst = out_qc[qc][:, (2 * p + h) * HD:(2 * p + h + 1) * HD]
                    src = banks[bnk][:, col + h * 65:col + h * 65 + 64]
                    if on_act:
                        nc.scalar.activation(out=dst, in_=src, func=COPY,
                                             scale=r[:, h:h + 1])
                    else:
                        nc.vector.tensor_scalar_mul(out=dst, in0=src,
                                                    scalar1=r[:, h:h + 1])

            def tpose(p, qc, on_act=False):
                tp = stgp.tile([128, 512], F32, tag="stage", name="stage")
                nc.tensor.transpose(
                    tp[:, 0:128], in_=out_qc[qc][:, p * 128:(p + 1) * 128],
                    identity=ident[:])
                dst = outT[p][:, qc * 128:(qc + 1) * 128]
                if on_act:
                    nc.scalar.copy(out=dst, in_=tp[:, 0:128])
                else:
                    nc.vector.tensor_copy(out=dst, in_=tp[:, 0:128])

            def clf_partial(qc, s0):
                # classifier contraction chunks 0..3 -> bf16 partial, runs
                # through the staging bank during pairs 4-5
                sw = min(512, NCLS - s0)
                ps = stgp.tile([128, 512], F32, tag="stage", name="stage")
                for c in range(4):
                    nc.tensor.matmul(
                        ps[:, 0:sw], lhsT=outT[c][:, qc * 128:(qc + 1) * 128],
                        rhs=wf[c][:, s0:s0 + sw],
                        start=(c == 0), stop=(c == 3))
                nc.vector.tensor_copy(out=pcl[qc][:, s0:s0 + sw], in_=ps[:, 0:sw])

            def clf_tail(qc):
                # partial re-enters PSUM via an identity copy-matmul, then
                # the last two contraction chunks accumulate on top
                ps = psp.tile([128, 1024], F32, tag="ps", name="ps")
                for s0 in (0, 512):
                    sw = min(512, NCLS - s0)
                    nc.tensor.matmul(ps[:, s0:s0 + sw], lhsT=identb[:],
                                     rhs=pcl[qc][:, s0:s0 + sw],
                                     start=True, stop=False)
                    for c in (4, 5):
                        nc.tensor.matmul(
                            ps[:, s0:s0 + sw],
                            lhsT=outT[c][:, qc * 128:(qc + 1) * 128],
                            rhs=wf[c][:, s0:s0 + sw],
                            start=False, stop=(c == 5))
                acc = lgmax[qc % 2]
                if qc < 2:
                    nc.vector.tensor_copy(out=acc[:], in_=ps[:, 0:NCLS])
                else:
                    nc.vector.tensor_max(out=acc[:], in0=ps[:, 0:NCLS],
                                         in1=acc[:])

            # ---- software-pipelined emission ----
            # extras[s]: filler callables interleaved at global step s
            extras = {}

            def add_extra(s, f):
                extras.setdefault(s, []).append(f)

            # pair 0's q + first-half k ride the two psp tiles before step 0
            # (their DMA inputs land first); the rest are fillers with slack.
            add_extra(5, lambda: k_half(0, 1024))
            add_extra(7, lambda: k_half(0, 1536))
            for i, f in enumerate(qk_halves(1)):
                add_extra(6 + 2 * i, f)
            for p in range(2, PAIRS):
                base = (p - 2) * KC
                for i, f in enumerate(qk_halves(p)):
                    add_extra(base + 3 + 5 * i, f)
            # v group 0 (pairs 0-1) rides inside pair 0 (consumed LAG steps
            # later); groups 1 and 2 spread over the two preceding pairs.
            for kc in range(KC):
                add_extra(kc + 1, lambda kc=kc: v_group(0, kc))
                add_extra(KC + 2 * kc, lambda kc=kc: v_group(1, kc))
                add_extra(3 * KC + 2 * kc, lambda kc=kc: v_group(2, kc))
            # classifier partials (contraction chunks 0..3) ride pairs 4-5
            # once pair 3's transposes have landed
            for i, (qc, s0) in enumerate(
                    (qc, s0) for qc in range(NQ // 128) for s0 in (0, 512)):
                add_extra(73 + (i * 22) // 16,
                          lambda qc=qc, s0=s0: clf_partial(qc, s0))

            qk_unit(0, "q", 0)
            qk_unit(0, "k", 0)

            # Emission order within a step: trailing attn@v first (deps long
            # satisfied, keeps the PE engine fed), then scores (these block
            # on the exp double-buffer rotation - the genuine rate limiter),
            # then fillers through the separate staging bank.
            def mark(label):
                # consumes one instruction name; records the exact next id
                EMIT_LOG.append((int(nc.get_next_instruction_name()[2:]), label))

            total = PAIRS * KC
            for s in range(total + LAG):
                t = s - LAG
                if t >= 0:
                    ap_, akc = divmod(t, KC)
                    mark(f"s{s}:av(p{ap_},kc{akc})")
                    av_step(ap_, akc)
                    if akc == KC - 1 and ap_ < PAIRS - 1:
                        mark(f"s{s}:norm(p{ap_})")
                        # norms gate the next pair's attn@v bank reuse (WAR);
                        # jump the DVE queue so the banks free promptly
                        with tc.high_priority():
                            for qc in range(NQ // 128):
                                norm(ap_, qc)
                        # spread the pair's transposes over upcoming steps
                        for qc in range(NQ // 128):
                            add_extra(s + 2 + qc // 2,
                                      lambda ap_=ap_, qc=qc: tpose(ap_, qc))
                if s < total:
                    p, kc = divmod(s, KC)
                    mark(f"s{s}:scores(p{p},kc{kc})")
                    scores_step(p, kc)
                mark(f"s{s}:extras")
                for f in extras.pop(s, ()):
                    f()

            # tail: last pair's normalize/transpose feeds the classifier
            # per query chunk so the max chains start as early as possible
            for qc in range(NQ // 128):
                norm(PAIRS - 1, qc, on_act=True)
                tpose(PAIRS - 1, qc, on_act=True)
                clf_tail(qc)
            nc.vector.tensor_max(out=lgmax[0][:], in0=lgmax[1][:],
                                 in1=lgmax[0][:])

            # final 128-way partition max happens on the host
            nc.sync.dma_start(out=out_d[:, :], in_=lgmax[0][:])

    nc.compile()
    return nc


def _prep_inputs(x, w_qkv, w_proj, b_proj, w_head, b_head):
    bf = ml_dtypes.bfloat16
    f8 = ml_dtypes.float8_e4m3fn
    x = np.asarray(x, dtype=np.float32)
    w_qkv = np.asarray(w_qkv, np.float32)
    wq8 = np.zeros((C, 2 * C), f8)
    wq8[:, 0:C] = (w_qkv[:, 0:C] * QK_PRESCALE).astype(f8)
    wk8 = np.zeros((C, 2 * C), f8)
    wk8[:, 0:C] = (w_qkv[:, C:2 * C] * QK_PRESCALE).astype(f8)
    wv_b = np.ascontiguousarray(w_qkv[:, 2 * C:].astype(bf))
    wfm = (np.asarray(w_proj, np.float64) @ np.asarray(w_head, np.float64))
    wf_b = np.ascontiguousarray(wfm.astype(np.float32).astype(bf))
    b_const = (np.asarray(b_proj, np.float32) @ np.asarray(w_head, np.float32)
               + np.asarray(b_head, np.float32))
    ident = np.eye(128, dtype=np.float32)
    identb = np.eye(128, dtype=bf)

    in_maps = []
    for core in range(8):
        b, half = core // 2, core % 2
        xb = x[b] if half == 0 else np.concatenate(
            [x[b, NQ:], x[b, :NQ]], axis=0)   # rotate keys: own queries first
        xTb = np.ascontiguousarray(xb.T.astype(bf))                # [768, 2048]
        xT8 = np.ascontiguousarray(xb.T.astype(f8))
        in_maps.append({"xT": xTb, "xT8": xT8, "wq8": wq8, "wk8": wk8,
                        "wv": wv_b, "wf": wf_b, "ident": ident,
                        "identb": identb})
    return in_maps, b_const


def kernel(x, w_qkv, w_proj, b_proj, w_head, b_head):
    if "nc" not in _CACHE:
        _CACHE["nc"] = _build()
    nc = _CACHE["nc"]

    in_maps, b_const = _prep_inputs(x, w_qkv, w_proj, b_proj, w_head, b_head)
    res = run_bass_kernel_spmd(nc, in_maps, core_ids=list(range(8)))

    out = np.empty((B, NUM_CLASSES), np.float32)
    for b in range(B):
        lo = res.results[2 * b]["out"].max(axis=0)
        hi = res.results[2 * b + 1]["out"].max(axis=0)
        out[b] = np.maximum(lo, hi)[:NUM_CLASSES] + b_const
    return out


if __name__ == "__main__":
    sys.path.insert(0, "/root/problem")
    import reference

    inputs = {k: np.asarray(v) for k, v in reference.setup_inputs().items()}
    expected = np.asarray(reference.reference(**inputs))
    actual = kernel(**inputs)
    num = np.linalg.norm(actual - expected)
    den = np.linalg.norm(expected)
    print("rel fro err:", num / den)
